# revision 9
# baseline (speedup 1.0000x reference)
"""Cross-attention kernel for 8 Trainium2 NeuronCores.

Reference computation (per batch element a, head i, full 256-dim per head):
  K_i = X @ Wk_i, Q_i = L @ Wq_i, V_i = X @ Wv_i
  S_i = Q_i @ K_i^T / sqrt(32); P = softmax(S); A_i = P_i @ V_i
  out = sum_i A_i @ Wu_i + bu

Host-side weight fusion (weights only, exact):
  M_i  = Wq_i @ Wk_i^T   =>  S_i = (L @ M_i) @ X^T     (K projection gone)
  W~_i = Wv_i @ Wu_i     =>  out = sum_i P_i @ (X @ W~_i)   (Wu matmul gone)

Sharding: core c = 2*a + hg handles batch a and head-group hg (4 heads).
The two partial outputs per batch element are summed on the host, which
also adds the bias.

Device data flow (per core, everything fp32r, contraction dim on SBUF
partitions so no on-device transposes):
  GT = (L @ MC)^T   [1024(g), 1024(y)]   via lhsT=MC-tile, rhs=LT
  U  = X @ WT       [1024(b), 4*258]     via lhsT=XT-tile, rhs=WT, evicted
                    with a stride that leaves a ones column after each head
  S^T_i [b,y] psum  via lhsT=XT-tile, rhs=GT(head i rows)
  P^T_i = exp(.)    [b,y] sbuf, ACT exp with 1/sqrt(32) scale fused
  out[y, 258] psum  += P^T_i-tile^T @ [U_i | 1 | pad]  over b-tiles; col 256
                    is the softmax denominator (ones-column trick), already
                    in [y-partition] orientation; col 257 pads to even N
                    (fp32r matmul ISA requires it)
  eviction          rec = 1/psum[:,256] (DVE, [128,1]); fused
                    osb = psum[:, :256] * rec + osb   (scalar_tensor_tensor)
  O [1024, 256]     natural row-major output, two 512-row DMAs

The ones-column trick removes the entire softmax-denominator reduction
(adder tree + partition_all_reduce + wide reciprocals) of the baseline,
which was stalling the PE ~11us per head-chunk. SBUF tiles are consolidated
into one tile per logical tensor to keep the tile-release epilogue short.
"""

import math
import sys

import numpy as np

sys.path.insert(0, "/opt/trn_rl_repo")

import concourse.bass as bass  # noqa: E402
import concourse.mybir as mybir  # noqa: E402
from concourse import bacc  # noqa: E402
from concourse.bass_utils import run_bass_kernel_spmd  # noqa: E402
from concourse.tile import TileContext  # noqa: E402

F32 = mybir.dt.float32
F32R = mybir.dt.float32r
EXP = mybir.ActivationFunctionType.Exp
COPY = mybir.ActivationFunctionType.Copy
MULT = mybir.AluOpType.mult
ADD = mybir.AluOpType.add

B, S, E = 4, 1024, 256          # batch, seq, embed
HEADS = 8                        # total heads; each head dim = E (source quirk)
N_CORES = 8
HG = 4                           # heads per head-group (per core)
NH = HG * E                      # fused-weight columns per core = 1024
SCALE = 1.0 / math.sqrt(E // HEADS)   # 1/sqrt(32)

P = 128                          # SBUF partitions
ET = E // P                      # 2 contraction tiles over embed
ST = S // P                      # 8 tiles over seq (b or y)
NCH = 512                        # matmul moving-dim chunk
EC = E + 2                       # 258: out + sums col + pad (fp32r needs even N)
UW = HG * EC                     # 1032: U width per b-tile (4 heads + extras)

_CACHE = {}


def _build():
    nc = bacc.Bacc(target_bir_lowering=False)

    XT = nc.dram_tensor("XT", [E, S], F32R, kind="ExternalInput")
    LT = nc.dram_tensor("LT", [E, S], F32R, kind="ExternalInput")
    MC = nc.dram_tensor("MC", [E, NH], F32R, kind="ExternalInput")
    WT = nc.dram_tensor("WT", [E, NH], F32R, kind="ExternalInput")
    O = nc.dram_tensor("O", [S, E], F32, kind="ExternalOutput")

    with TileContext(nc) as tc:
        with tc.tile_pool(name="persist", bufs=1) as pp, \
             tc.tile_pool(name="psum", bufs=1, space="PSUM") as ps:

            # one consolidated SBUF tile per logical tensor; sliced views
            xt_a = pp.tile([P, ET * S], F32R, tag="xt", name="xt")
            lt_a = pp.tile([P, ET * S], F32R, tag="lt", name="lt")
            mc_a = pp.tile([P, ET * S], F32R, tag="mc", name="mc")
            wt_a = pp.tile([P, ET * S], F32R, tag="wt", name="wt")
            gt_a = pp.tile([P, ST * S], F32R, tag="gt", name="gt")
            u_a = pp.tile([P, ST * UW], F32R, tag="u", name="u")
            pt_a = [pp.tile([P, ST * S], F32R, tag=f"pt{s}", name=f"pt{s}")
                    for s in range(2)]
            osb_a = pp.tile([P, ST * E], F32, tag="osb", name="osb")
            rc_a = pp.tile([P, 2 * ST], F32, tag="rc", name="rc")

            xt = [xt_a[:, e * S:(e + 1) * S] for e in range(ET)]
            lt = [lt_a[:, e * S:(e + 1) * S] for e in range(ET)]
            mc = [mc_a[:, e * S:(e + 1) * S] for e in range(ET)]
            wt = [wt_a[:, e * S:(e + 1) * S] for e in range(ET)]
            gt = [gt_a[:, g * S:(g + 1) * S] for g in range(ST)]
            u = [u_a[:, bt * UW:(bt + 1) * UW] for bt in range(ST)]
            pt = [[pt_a[s][:, bt * S:(bt + 1) * S] for bt in range(ST)]
                  for s in range(2)]
            osb = [osb_a[:, yt * E:(yt + 1) * E] for yt in range(ST)]
            rc = [rc_a[:, s * ST:(s + 1) * ST] for s in range(2)]

            # ---- input DMA: one instruction covers both e-tiles of a
            #      column range; ordered so compute can start early ----
            def dma_in(dst_a, dram, c0, c1):
                nc.sync.dma_start(
                    out=dst_a[:].rearrange("p (e s) -> p e s", e=ET)[:, :, c0:c1],
                    in_=dram.rearrange("(e p) s -> p e s", p=P)[:, :, c0:c1])

            dma_in(mc_a, MC, 0, 256)             # g0-g1 lhsT
            dma_in(lt_a, LT, 0, NCH)             # chunk-0 rhs
            dma_in(mc_a, MC, 256, NCH)           # g2-g3
            dma_in(lt_a, LT, NCH, S)             # chunk-1 rhs
            dma_in(mc_a, MC, NCH, S)             # g4-g7
            dma_in(xt_a, XT, 0, NCH)
            dma_in(xt_a, XT, NCH, S)
            dma_in(wt_a, WT, 0, S)

            # ---- PE warm-up: tiny self-contained matmuls keep the PE busy
            #      through the DMA wait so DVFS is at max clock (2.4 GHz)
            #      when the real stream starts (ramp takes ~3us) ----
            wsc = pp.tile([P, 16], F32R, tag="wsc", name="wsc")
            nc.vector.memset(wsc[:].bitcast(F32), 1.0)
            for w in range(48):
                wp = ps.tile([P, NCH], F32, tag="sc", bufs=4, name=f"wp{w}")
                nc.tensor.matmul(wp[0:2, 0:16], wsc[:, 0:2], wsc[:],
                                 start=True, stop=True)

            # ones + pad columns of the U blocks (cols 256,257 per head block)
            for bt in range(ST):
                for h in range(HG):
                    nc.gpsimd.memset(
                        u[bt][:, h * EC + E:(h + 1) * EC].bitcast(F32), 1.0)

            # alternate PSUM->SBUF evictions between DVE and ACT
            # (GPSIMD cannot access PSUM on TRN2)
            ev_ctr = [0]

            def evict(dst_ap, src_ap):
                ev_ctr[0] += 1
                if ev_ctr[0] % 2 == 0:
                    nc.vector.tensor_copy(dst_ap, src_ap)
                else:
                    nc.scalar.activation(dst_ap, src_ap, COPY)

            # ---- GT projection: GT[g,y] = sum_e MC[e,g] * LT[e,y] ----
            def gt_proj(g, c):
                sl = bass.ts(c, NCH)
                pg = ps.tile([P, NCH], F32, tag="sc", bufs=4, name=f"pg{g}{c}")
                for e in range(ET):
                    nc.tensor.matmul(pg[:], mc[e][:, g * P:(g + 1) * P],
                                     lt[e][:, sl],
                                     start=(e == 0), stop=(e == ET - 1))
                evict(gt[g][:, sl], pg[:])

            # ---- U projection: U[b,g] = sum_e XT[e,b] * WT[e,g], strided
            #      eviction skips the ones/pad columns of each head block ----
            def u_proj(bt, gc):
                pu = ps.tile([P, NCH], F32, tag="sc", bufs=4, name=f"pu{bt}{gc}")
                for e in range(ET):
                    nc.tensor.matmul(pu[:], xt[e][:, bt * P:(bt + 1) * P],
                                     wt[e][:, gc * NCH:(gc + 1) * NCH],
                                     start=(e == 0), stop=(e == ET - 1))
                uv = u[bt].rearrange("p (h x) -> p h x", h=HG)
                pv = pu[:].rearrange("p (h x) -> p h x", h=2)
                evict(uv[:, 2 * gc:2 * gc + 2, 0:E], pv[:])

            # ---- scores + exp for head h, chunk c ----
            def scores(h, c):
                sl = bass.ts(c, NCH)
                pts = pt[h % 2]
                for bt in range(ST):
                    pss = ps.tile([P, NCH], F32, tag="sc", bufs=4,
                                  name=f"pss{h}{c}{bt}")
                    for e in range(ET):
                        nc.tensor.matmul(pss[:], xt[e][:, bt * P:(bt + 1) * P],
                                         gt[2 * h + e][:, sl],
                                         start=(e == 0), stop=(e == ET - 1))
                    nc.scalar.activation(pts[bt][:, sl], pss[:], EXP, scale=SCALE)

            # ---- out accumulation for head h, y-tiles of chunk c;
            #      psum col 256 = softmax denominator; fused normalize ----
            def outq(h, c):
                pts = pt[h % 2]
                r = rc[h % 2]
                for yt in range(4 * c, 4 * c + 4):
                    po = ps.tile([P, EC], F32, tag="po", bufs=4,
                                 name=f"po{h}{yt}")
                    for bt in range(ST):
                        nc.tensor.matmul(po[:], pts[bt][:, yt * P:(yt + 1) * P],
                                         u[bt][:, h * EC:(h + 1) * EC],
                                         start=(bt == 0), stop=(bt == ST - 1))
                    nc.vector.reciprocal(r[:, yt:yt + 1], po[:, E:E + 1])
                    if h == 0:
                        nc.vector.tensor_scalar_mul(osb[yt], po[:, 0:E],
                                                    r[:, yt:yt + 1])
                    else:
                        nc.vector.scalar_tensor_tensor(osb[yt], po[:, 0:E],
                                                       r[:, yt:yt + 1],
                                                       osb[yt], MULT, ADD)

            def dma_out(q):
                # quarter-output DMA (2 y-tiles); range-level deps on writers
                piece = osb_a[:, 2 * q * E:(2 * q + 2) * E]
                nc.sync.dma_start(
                    out=O[2 * q * P:(2 * q + 2) * P, :].rearrange(
                        "(t p) e -> p t e", p=P),
                    in_=piece.rearrange("p (t e) -> p t e", t=2))

            # ---- PE program order: keep the PE busy while ACT drains exps ----
            for g in range(4):
                gt_proj(g, 0)
            for g in range(4):
                gt_proj(g, 1)
            for g in range(4, ST):
                gt_proj(g, 0)
            for g in range(4, ST):
                gt_proj(g, 1)
            scores(0, 0)
            scores(0, 1)
            for bt in range(ST):
                for gc in range(2):
                    u_proj(bt, gc)
            scores(1, 0)
            scores(1, 1)
            outq(0, 0)
            outq(0, 1)
            scores(2, 0)
            scores(2, 1)
            outq(1, 0)
            outq(1, 1)
            scores(3, 0)
            scores(3, 1)
            outq(2, 0)
            outq(2, 1)
            outq(3, 0)
            dma_out(0)
            dma_out(1)
            outq(3, 1)
            dma_out(2)
            dma_out(3)

    nc.compile()
    return nc


def kernel(batch, latent, Wk, Wq, Wv, Wu, bu):
    batch = np.asarray(batch, dtype=np.float32)
    latent = np.asarray(latent, dtype=np.float32)
    Wk = np.asarray(Wk, dtype=np.float32)
    Wq = np.asarray(Wq, dtype=np.float32)
    Wv = np.asarray(Wv, dtype=np.float32)
    Wu = np.asarray(Wu, dtype=np.float32)
    bu = np.asarray(bu, dtype=np.float32)

    if "nc" not in _CACHE:
        _CACHE["nc"] = _build()
    nc = _CACHE["nc"]

    in_maps = []
    for core in range(N_CORES):
        a, hg = core // 2, core % 2
        mcs, wts = [], []
        for j in range(HG):
            i = hg * HG + j                      # global head index
            cols = slice(i * E, (i + 1) * E)
            mcs.append(Wq[:, cols] @ Wk[:, cols].T)
            wts.append(Wv[:, cols] @ Wu[cols.start:cols.stop, :])
        in_maps.append({
            "XT": np.ascontiguousarray(batch[a].T),
            "LT": np.ascontiguousarray(latent[a].T),
            "MC": np.ascontiguousarray(np.concatenate(mcs, axis=1)),
            "WT": np.ascontiguousarray(np.concatenate(wts, axis=1)),
        })

    _CACHE["in_maps"] = in_maps
    res = run_bass_kernel_spmd(nc, in_maps, core_ids=list(range(N_CORES)))

    out = np.empty((B, S, E), dtype=np.float32)
    for a in range(B):
        out[a] = res.results[2 * a]["O"] + res.results[2 * a + 1]["O"] + bu
    return out


# revision 10
# speedup vs baseline: 1.0084x; 1.0084x over previous
"""Cross-attention kernel for 8 Trainium2 NeuronCores.

Reference computation (per batch element a, head i, full 256-dim per head):
  K_i = X @ Wk_i, Q_i = L @ Wq_i, V_i = X @ Wv_i
  S_i = Q_i @ K_i^T / sqrt(32); P = softmax(S); A_i = P_i @ V_i
  out = sum_i A_i @ Wu_i + bu

Host-side weight fusion (weights only, exact):
  M_i  = Wq_i @ Wk_i^T   =>  S_i = (L @ M_i) @ X^T     (K projection gone)
  W~_i = Wv_i @ Wu_i     =>  out = sum_i P_i @ (X @ W~_i)   (Wu matmul gone)

Sharding: core c = 2*a + hg handles batch a and head-group hg (4 heads).
The two partial outputs per batch element are summed on the host, which
also adds the bias.

Device data flow (per core, everything fp32r, contraction dim on SBUF
partitions so no on-device transposes):
  GT = (L @ MC)^T   [1024(g), 1024(y)]   via lhsT=MC-tile, rhs=LT
  U  = X @ WT       [1024(b), 4*258]     via lhsT=XT-tile, rhs=WT, evicted
                    with a stride that leaves a ones column after each head
  S^T_i [b,y] psum  via lhsT=XT-tile, rhs=GT(head i rows)
  P^T_i = exp(.)    [b,y] sbuf, ACT exp with 1/sqrt(32) scale fused
  out[y, 258] psum  += P^T_i-tile^T @ [U_i | 1 | pad]  over b-tiles; col 256
                    is the softmax denominator (ones-column trick), already
                    in [y-partition] orientation; col 257 pads to even N
                    (fp32r matmul ISA requires it)
  eviction          rec = 1/psum[:,256] (DVE, [128,1]); fused
                    osb = psum[:, :256] * rec + osb   (scalar_tensor_tensor)
  O [1024, 256]     natural row-major output, two 512-row DMAs

The ones-column trick removes the entire softmax-denominator reduction
(adder tree + partition_all_reduce + wide reciprocals) of the baseline,
which was stalling the PE ~11us per head-chunk. SBUF tiles are consolidated
into one tile per logical tensor to keep the tile-release epilogue short.
"""

import math
import sys

import numpy as np

sys.path.insert(0, "/opt/trn_rl_repo")

import concourse.bass as bass  # noqa: E402
import concourse.mybir as mybir  # noqa: E402
from concourse import bacc  # noqa: E402
from concourse.bass_utils import run_bass_kernel_spmd  # noqa: E402
from concourse.tile import TileContext  # noqa: E402

F32 = mybir.dt.float32
F32R = mybir.dt.float32r
EXP = mybir.ActivationFunctionType.Exp
COPY = mybir.ActivationFunctionType.Copy
MULT = mybir.AluOpType.mult
ADD = mybir.AluOpType.add

B, S, E = 4, 1024, 256          # batch, seq, embed
HEADS = 8                        # total heads; each head dim = E (source quirk)
N_CORES = 8
HG = 4                           # heads per head-group (per core)
NH = HG * E                      # fused-weight columns per core = 1024
SCALE = 1.0 / math.sqrt(E // HEADS)   # 1/sqrt(32)

P = 128                          # SBUF partitions
ET = E // P                      # 2 contraction tiles over embed
ST = S // P                      # 8 tiles over seq (b or y)
NCH = 512                        # matmul moving-dim chunk
EC = E + 2                       # 258: out + sums col + pad (fp32r needs even N)
UW = HG * EC                     # 1032: U width per b-tile (4 heads + extras)

_CACHE = {}


def _build():
    nc = bacc.Bacc(target_bir_lowering=False)

    XT = nc.dram_tensor("XT", [E, S], F32R, kind="ExternalInput")
    LT = nc.dram_tensor("LT", [E, S], F32R, kind="ExternalInput")
    MC = nc.dram_tensor("MC", [E, NH], F32R, kind="ExternalInput")
    WT = nc.dram_tensor("WT", [E, NH], F32R, kind="ExternalInput")
    O = nc.dram_tensor("O", [S, E], F32, kind="ExternalOutput")

    with TileContext(nc) as tc:
        with tc.tile_pool(name="persist", bufs=1) as pp, \
             tc.tile_pool(name="psum", bufs=1, space="PSUM") as ps:

            # one consolidated SBUF tile per logical tensor; sliced views
            xt_a = pp.tile([P, ET * S], F32R, tag="xt", name="xt")
            lt_a = pp.tile([P, ET * S], F32R, tag="lt", name="lt")
            mc_a = pp.tile([P, ET * S], F32R, tag="mc", name="mc")
            wt_a = pp.tile([P, ET * S], F32R, tag="wt", name="wt")
            gt_a = pp.tile([P, ST * S], F32R, tag="gt", name="gt")
            u_a = pp.tile([P, ST * UW], F32R, tag="u", name="u")
            pt_a = [pp.tile([P, ST * S], F32R, tag=f"pt{s}", name=f"pt{s}")
                    for s in range(2)]
            osb_a = pp.tile([P, ST * E], F32, tag="osb", name="osb")
            rc_a = pp.tile([P, 2 * ST], F32, tag="rc", name="rc")

            xt = [xt_a[:, e * S:(e + 1) * S] for e in range(ET)]
            lt = [lt_a[:, e * S:(e + 1) * S] for e in range(ET)]
            mc = [mc_a[:, e * S:(e + 1) * S] for e in range(ET)]
            wt = [wt_a[:, e * S:(e + 1) * S] for e in range(ET)]
            gt = [gt_a[:, g * S:(g + 1) * S] for g in range(ST)]
            u = [u_a[:, bt * UW:(bt + 1) * UW] for bt in range(ST)]
            pt = [[pt_a[s][:, bt * S:(bt + 1) * S] for bt in range(ST)]
                  for s in range(2)]
            osb = [osb_a[:, yt * E:(yt + 1) * E] for yt in range(ST)]
            rc = [rc_a[:, s * ST:(s + 1) * ST] for s in range(2)]

            # ---- input DMA: per-e-tile 2D transfers (a merged 3D AP would
            #      collapse the 16-way DMA-engine fan-out to 2-way), ordered
            #      so compute can start early ----
            def dma_in(dst, dram, e, c0, c1):
                nc.sync.dma_start(out=dst[e][:, c0:c1],
                                  in_=dram[e * P:(e + 1) * P, c0:c1])

            for e in range(ET):
                dma_in(mc, MC, e, 0, 256)        # g0-g1 lhsT
            for e in range(ET):
                dma_in(lt, LT, e, 0, NCH)        # chunk-0 rhs
            for e in range(ET):
                dma_in(mc, MC, e, 256, NCH)      # g2-g3
            for e in range(ET):
                dma_in(mc, MC, e, NCH, S)        # g4-g7
            for e in range(ET):
                dma_in(lt, LT, e, NCH, S)        # chunk-1 rhs
            for c in range(2):
                for e in range(ET):
                    dma_in(xt, XT, e, c * NCH, (c + 1) * NCH)
            for e in range(ET):
                dma_in(wt, WT, e, 0, S)

            # ---- PE warm-up: tiny self-contained matmuls keep the PE busy
            #      through the DMA wait so DVFS is at max clock (2.4 GHz,
            #      ~3us continuous-busy ramp) when the real stream starts ----
            wsc = pp.tile([P, 16], F32R, tag="wsc", name="wsc")
            nc.vector.memset(wsc[:].bitcast(F32), 1.0)
            for w in range(80):
                wp = ps.tile([P, NCH], F32, tag="sc", bufs=4, name=f"wp{w}")
                nc.tensor.matmul(wp[0:2, 0:16], wsc[:, 0:2], wsc[:],
                                 start=True, stop=True)

            # ones + pad columns of the U blocks (cols 256,257 per head block)
            for bt in range(ST):
                for h in range(HG):
                    nc.gpsimd.memset(
                        u[bt][:, h * EC + E:(h + 1) * EC].bitcast(F32), 1.0)

            # alternate PSUM->SBUF evictions between DVE and ACT
            # (GPSIMD cannot access PSUM on TRN2)
            ev_ctr = [0]

            def evict(dst_ap, src_ap):
                ev_ctr[0] += 1
                if ev_ctr[0] % 2 == 0:
                    nc.vector.tensor_copy(dst_ap, src_ap)
                else:
                    nc.scalar.activation(dst_ap, src_ap, COPY)

            # ---- GT projection: GT[g,y] = sum_e MC[e,g] * LT[e,y] ----
            def gt_proj(g, c):
                sl = bass.ts(c, NCH)
                pg = ps.tile([P, NCH], F32, tag="sc", bufs=4, name=f"pg{g}{c}")
                for e in range(ET):
                    nc.tensor.matmul(pg[:], mc[e][:, g * P:(g + 1) * P],
                                     lt[e][:, sl],
                                     start=(e == 0), stop=(e == ET - 1))
                evict(gt[g][:, sl], pg[:])

            # ---- U projection: U[b,g] = sum_e XT[e,b] * WT[e,g], strided
            #      eviction skips the ones/pad columns of each head block ----
            def u_proj(bt, gc):
                pu = ps.tile([P, NCH], F32, tag="sc", bufs=4, name=f"pu{bt}{gc}")
                for e in range(ET):
                    nc.tensor.matmul(pu[:], xt[e][:, bt * P:(bt + 1) * P],
                                     wt[e][:, gc * NCH:(gc + 1) * NCH],
                                     start=(e == 0), stop=(e == ET - 1))
                uv = u[bt].rearrange("p (h x) -> p h x", h=HG)
                pv = pu[:].rearrange("p (h x) -> p h x", h=2)
                evict(uv[:, 2 * gc:2 * gc + 2, 0:E], pv[:])

            # ---- scores + exp for head h, chunk c ----
            def scores(h, c):
                sl = bass.ts(c, NCH)
                pts = pt[h % 2]
                for bt in range(ST):
                    pss = ps.tile([P, NCH], F32, tag="sc", bufs=4,
                                  name=f"pss{h}{c}{bt}")
                    for e in range(ET):
                        nc.tensor.matmul(pss[:], xt[e][:, bt * P:(bt + 1) * P],
                                         gt[2 * h + e][:, sl],
                                         start=(e == 0), stop=(e == ET - 1))
                    nc.scalar.activation(pts[bt][:, sl], pss[:], EXP, scale=SCALE)

            # ---- out accumulation for head h, y-tiles of chunk c;
            #      psum col 256 = softmax denominator; fused normalize ----
            def outq(h, c):
                pts = pt[h % 2]
                r = rc[h % 2]
                for yt in range(4 * c, 4 * c + 4):
                    po = ps.tile([P, EC], F32, tag="po", bufs=4,
                                 name=f"po{h}{yt}")
                    for bt in range(ST):
                        nc.tensor.matmul(po[:], pts[bt][:, yt * P:(yt + 1) * P],
                                         u[bt][:, h * EC:(h + 1) * EC],
                                         start=(bt == 0), stop=(bt == ST - 1))
                    nc.vector.reciprocal(r[:, yt:yt + 1], po[:, E:E + 1])
                    if h == 0:
                        nc.vector.tensor_scalar_mul(osb[yt], po[:, 0:E],
                                                    r[:, yt:yt + 1])
                    else:
                        nc.vector.scalar_tensor_tensor(osb[yt], po[:, 0:E],
                                                       r[:, yt:yt + 1],
                                                       osb[yt], MULT, ADD)
                    if h == HG - 1:
                        nc.sync.dma_start(out=O[yt * P:(yt + 1) * P, :],
                                          in_=osb[yt])


            # ---- PE program order: keep the PE busy while ACT drains exps ----
            for g in range(ST):
                gt_proj(g, 0)
            for g in range(ST):
                gt_proj(g, 1)
            scores(0, 0)
            scores(0, 1)
            for bt in range(ST):
                for gc in range(2):
                    u_proj(bt, gc)
            scores(1, 0)
            scores(1, 1)
            outq(0, 0)
            outq(0, 1)
            scores(2, 0)
            scores(2, 1)
            outq(1, 0)
            outq(1, 1)
            scores(3, 0)
            scores(3, 1)
            outq(2, 0)
            outq(2, 1)
            outq(3, 0)
            outq(3, 1)

    nc.compile()
    return nc


def kernel(batch, latent, Wk, Wq, Wv, Wu, bu):
    batch = np.asarray(batch, dtype=np.float32)
    latent = np.asarray(latent, dtype=np.float32)
    Wk = np.asarray(Wk, dtype=np.float32)
    Wq = np.asarray(Wq, dtype=np.float32)
    Wv = np.asarray(Wv, dtype=np.float32)
    Wu = np.asarray(Wu, dtype=np.float32)
    bu = np.asarray(bu, dtype=np.float32)

    if "nc" not in _CACHE:
        _CACHE["nc"] = _build()
    nc = _CACHE["nc"]

    in_maps = []
    for core in range(N_CORES):
        a, hg = core // 2, core % 2
        mcs, wts = [], []
        for j in range(HG):
            i = hg * HG + j                      # global head index
            cols = slice(i * E, (i + 1) * E)
            mcs.append(Wq[:, cols] @ Wk[:, cols].T)
            wts.append(Wv[:, cols] @ Wu[cols.start:cols.stop, :])
        in_maps.append({
            "XT": np.ascontiguousarray(batch[a].T),
            "LT": np.ascontiguousarray(latent[a].T),
            "MC": np.ascontiguousarray(np.concatenate(mcs, axis=1)),
            "WT": np.ascontiguousarray(np.concatenate(wts, axis=1)),
        })

    _CACHE["in_maps"] = in_maps
    res = run_bass_kernel_spmd(nc, in_maps, core_ids=list(range(N_CORES)))

    out = np.empty((B, S, E), dtype=np.float32)
    for a in range(B):
        out[a] = res.results[2 * a]["O"] + res.results[2 * a + 1]["O"] + bu
    return out
